# revision 9
# baseline (speedup 1.0000x reference)
"""Trainium2 Bass kernel for a dense transformer block (LN-attn-LN-MLP).

Sharding: 8 cores = (4 batches) x (2 query-halves). Each core computes k/v for
its batch's full 2048 tokens (duplicated within the pair; avoids collectives)
and queries/MLP for its own 1024 tokens. All activations are kept
feature-major ([C, T]) end-to-end so no on-chip transposes are needed; the
host passes x pre-transposed and un-transposes the output.
"""
import sys
sys.path.insert(0, "/opt/trn_rl_repo")

import numpy as np
import ml_dtypes

import concourse.bass as bass
import concourse.tile as tile
from concourse import bacc
from concourse import mybir
from concourse.bass_utils import run_bass_kernel_spmd

F32 = mybir.dt.float32
F32R = mybir.dt.float32r
BF16 = mybir.dt.bfloat16
AF = mybir.ActivationFunctionType
OP = mybir.AluOpType

P = 128
C = 768            # embed dim
CO = 6             # C / 128 chunks
H = 12             # heads
D = 64             # head dim
HID = 3072
HF = 24            # HID / 128 chunks
TK = 2048          # tokens per batch (keys/values)
TQ = 1024          # query tokens per core
NKT = TK // P      # 16 key tiles
NTC = TK // 512    # 4 token chunks (LN1)
NQC = TQ // 512    # 2 query chunks
LN_EPS = 1e-6
NPAIR = 6          # head pairs


def _build_nc(flags):
    """Build the single-core Bass program (identical for all 8 cores)."""
    nc = bacc.Bacc()

    xT_d = nc.declare_dram_parameter("xT", [C, TK], F32, isOutput=False)
    wqk_d = nc.declare_dram_parameter("wqk", [12, P, CO, P], BF16, isOutput=False)
    wv_d = nc.declare_dram_parameter("wv", [P, CO, C], BF16, isOutput=False)
    pjw_d = nc.declare_dram_parameter("pjw", [CO, P, CO, P], BF16, isOutput=False)
    f1w_d = nc.declare_dram_parameter("f1w", [HF, P, CO, P], BF16, isOutput=False)
    f2w_d = nc.declare_dram_parameter("f2w", [CO, P, HF, P], BF16, isOutput=False)
    qkb_d = nc.declare_dram_parameter("qkb", [P, 12], F32, isOutput=False)
    pjb_d = nc.declare_dram_parameter("pjb", [P, CO], F32, isOutput=False)
    f1b_d = nc.declare_dram_parameter("f1b", [P, HF], F32, isOutput=False)
    f2b_d = nc.declare_dram_parameter("f2b", [P, CO], F32, isOutput=False)
    if flags["ln1_aff"]:
        l1g_d = nc.declare_dram_parameter("l1g", [P, CO], F32, isOutput=False)
        l1b_d = nc.declare_dram_parameter("l1b", [P, CO], F32, isOutput=False)
    if flags["ln2_aff"]:
        l2g_d = nc.declare_dram_parameter("l2g", [P, CO], F32, isOutput=False)
        l2b_d = nc.declare_dram_parameter("l2b", [P, CO], F32, isOutput=False)
    if flags["vbias"]:
        vbb_d = nc.declare_dram_parameter("vbb", [1, 12 * 65], F32, isOutput=False)
    if flags["mask"]:
        mq_d = nc.declare_dram_parameter("mq", [1, TQ], F32, isOutput=False)
    out_d = nc.declare_dram_parameter("outT", [C, TQ], F32, isOutput=True)

    # internal DRAM spill for q^T / k^T (read back per head-pair)
    kTd = nc.dram_tensor("kT_spill", [NPAIR, P, TK], BF16)
    qTd = nc.dram_tensor("qT_spill", [NPAIR, P, TQ], BF16)

    xT3 = xT_d.rearrange("(co ci) t -> ci co t", ci=P)

    from contextlib import ExitStack
    with tile.TileContext(nc) as tc, ExitStack() as ctx:
        pool = lambda name, bufs, **kw: ctx.enter_context(
            tc.tile_pool(name=name, bufs=bufs, **kw))
        px = pool("px", 2)          # x6 chunks [128,6,512] f32
        psq = pool("psq", 2)        # squares [128,512] bf16
        ph1 = pool("ph1", 26)       # h1 then hid [128,512] bf16
        pw = pool("pw", 3)          # [128,6,128] bf16 weight tiles
        pwb = pool("pwb", 2)        # wv / fc2 big weight tiles
        pkq = pool("pkq", 2)        # kT per-pair [128,2048] bf16; outT reuse
        pqt = pool("pqt", 2)        # qT per-pair [128,1024] bf16
        pvt = pool("pvt", 16)       # v_aug [128,780] bf16
        pvl = pool("pvl", 6)        # vals [128,1024] bf16
        px2 = pool("px2", 6)        # x2 [128,1024] bf16
        pat = pool("pat", 4)        # attn exp / qkv epilogue [128,1024] bf16
        pxy = pool("pxy", 3)        # xmy [128,512] f32
        ph2 = pool("ph2", 12)       # h2 [128,512] bf16
        pst = pool("pst", 4)        # stat bcast [128,512] f32
        pu = pool("pu", 2)          # LN centered [128,512] f32
        prow = pool("prow", 4)      # [1,512] rows f32
        prb = pool("prb", 2)        # recip bcast [64,512] f32
        pout = pkq                  # outT shares the kq slots
        pstage = pat
        pone = pool("pone", 1)
        psc = pool("psc", 3, space="PSUM")  # [128,1024] 2-bank
        ppv = pool("ppv", 2, space="PSUM")  # 1-bank
        if True:
            ones_b = pone.tile([P, 1], BF16, tag="ones_b")
            nc.vector.memset(ones_b, 1.0)
            eps_sb = pone.tile([P, 1], F32, tag="eps")
            nc.vector.memset(eps_sb, LN_EPS)

            _bcn = [0]

            def bcast(out_ap, row_ap, npart, width):
                """out[0:npart, 0:width] = row broadcast across partitions.
                SBUF APs cannot have partition-step 0, so bounce the row
                through DRAM and broadcast-read it back (two DMAs, no PE)."""
                _bcn[0] += 1
                drow = nc.dram_tensor(f"bcrow{_bcn[0]}", [1, width], row_ap.dtype)
                nc.sync.dma_start(out=drow[:, :], in_=row_ap[0:1, 0:width])
                src = drow[0:1, 0:width]
                bap = bass.AP(tensor=src.tensor, offset=src.offset,
                              ap=[[0, npart]] + list(src.ap[1:]))
                nc.sync.dma_start(out=out_ap[0:npart, 0:width], in_=bap)

            qkb = pone.tile([P, 12], F32, tag="qkb")
            nc.sync.dma_start(out=qkb, in_=qkb_d[:, :])
            pjb = pone.tile([P, CO], F32, tag="pjb")
            nc.sync.dma_start(out=pjb, in_=pjb_d[:, :])
            f1b = pone.tile([P, HF], F32, tag="f1b")
            nc.sync.dma_start(out=f1b, in_=f1b_d[:, :])
            f2b = pone.tile([P, CO], F32, tag="f2b")
            nc.sync.dma_start(out=f2b, in_=f2b_d[:, :])
            if flags["ln1_aff"]:
                l1g = pone.tile([P, CO], F32, tag="l1g")
                nc.sync.dma_start(out=l1g, in_=l1g_d[:, :])
                l1b = pone.tile([P, CO], F32, tag="l1b")
                nc.sync.dma_start(out=l1b, in_=l1b_d[:, :])
            if flags["ln2_aff"]:
                l2g = pone.tile([P, CO], F32, tag="l2g")
                nc.sync.dma_start(out=l2g, in_=l2g_d[:, :])
                l2b = pone.tile([P, CO], F32, tag="l2b")
                nc.sync.dma_start(out=l2b, in_=l2b_d[:, :])
            if flags["vbias"]:
                vbrow = pone.tile([1, 12 * 65], F32, tag="vbrow")
                nc.sync.dma_start(out=vbrow, in_=vbb_d[:, :])
                vbrow_b = pone.tile([1, 12 * 65], BF16, tag="vbrow_b")
                nc.vector.tensor_copy(vbrow_b, vbrow)
                vbb = pone.tile([P, 12 * 65], BF16, tag="vbb")
                bcast(vbb, vbrow_b, P, 12 * 65)
            if flags["mask"]:
                mqrow = pone.tile([1, TQ], F32, tag="mqrow")
                nc.sync.dma_start(out=mqrow, in_=mq_d[:, :])
                mqrow_b = pone.tile([1, TQ], BF16, tag="mqrow_b")
                nc.vector.tensor_copy(mqrow_b, mqrow)
                mqb = pone.tile([P, TQ], BF16, tag="mqb")
                bcast(mqb, mqrow_b, P, TQ)

            wv_sb = pwb.tile([P, CO, C], BF16, tag="wbig")
            nc.sync.dma_start(out=wv_sb, in_=wv_d[:, :, :])

            # ---------------- LN1 (feature-major, per 512-token chunk) ------
            h1 = {}  # (co, tchunk 0..3) -> [128,512] bf16
            x6_keep = {}
            for t in range(NTC):
                x6 = px.tile([P, CO, 512], F32, tag="x6")
                nc.sync.dma_start(out=x6, in_=xT3[:, :, t * 512:(t + 1) * 512])
                mu_ps = ppv.tile([1, 512], F32, tag="pv")
                m2_ps = ppv.tile([1, 512], F32, tag="pv")
                for co in range(CO):
                    xb = psq.tile([P, 512], BF16, tag="xb")
                    nc.vector.tensor_copy(xb, x6[:, co, :])
                    sq = psq.tile([P, 512], BF16, tag="sq")
                    nc.vector.tensor_tensor(sq, xb, xb, OP.mult)
                    nc.tensor.matmul(mu_ps, ones_b[:, :], xb[:, :],
                                     start=(co == 0), stop=(co == CO - 1))
                    nc.tensor.matmul(m2_ps, ones_b[:, :], sq[:, :],
                                     start=(co == 0), stop=(co == CO - 1))
                mu_row = prow.tile([1, 512], BF16, tag="row")
                nc.vector.tensor_scalar_mul(mu_row, mu_ps, 1.0 / C)
                musq = prow.tile([1, 512], F32, tag="row")
                nc.vector.tensor_tensor(musq, mu_row, mu_row, OP.mult)
                var_row = prow.tile([1, 512], F32, tag="row")
                nc.vector.scalar_tensor_tensor(
                    out=var_row, in0=m2_ps, scalar=1.0 / C, in1=musq,
                    op0=OP.mult, op1=OP.subtract)
                sd_row = prow.tile([1, 512], F32, tag="row")
                nc.scalar.activation(out=sd_row, in_=var_row, func=AF.Sqrt,
                                     bias=eps_sb[0:1, :])
                rs_row = prow.tile([1, 512], F32, tag="row")
                nc.vector.reciprocal(out=rs_row, in_=sd_row)
                mu_b = pst.tile([P, 512], BF16, tag="st")
                bcast(mu_b, mu_row, P, 512)
                rs_b = pst.tile([P, 512], F32, tag="st")
                bcast(rs_b, rs_row, P, 512)
                for co in range(CO):
                    u = pu.tile([P, 512], F32, tag="u")
                    nc.vector.tensor_tensor(u, x6[:, co, :], mu_b, OP.subtract)
                    ht = ph1.tile([P, 512], BF16, tag="h1")
                    if flags["ln1_aff"]:
                        nc.vector.tensor_tensor(u, u, rs_b, OP.mult)
                        nc.vector.tensor_scalar(
                            out=ht, in0=u, scalar1=l1g[:, co:co + 1],
                            scalar2=l1b[:, co:co + 1], op0=OP.mult, op1=OP.add)
                    else:
                        nc.vector.tensor_tensor(ht, u, rs_b, OP.mult)
                    h1[(co, t)] = ht
                x6_keep[t] = x6

            # ---------------- QKV projections ------------------------------
            # q^T / k^T: weight-stationary, feature-major output -> DRAM spill
            for f in range(12):
                is_q = f < 6
                ntp = 1 if is_q else 2   # q: only my 1024 tokens
                wt = pw.tile([P, CO, P], BF16, tag="w")
                nc.sync.dma_start(out=wt, in_=wqk_d[f])
                for tp in range(ntp):
                    ps = psc.tile([P, 1024], F32, tag="sc")
                    for co in range(CO):
                        for th in range(2):
                            nc.tensor.matmul(
                                ps[:, th * 512:(th + 1) * 512], wt[:, co, :],
                                h1[(co, tp * 2 + th)][:, :],
                                start=(co == 0), stop=(co == CO - 1))
                    st = pstage.tile([P, 1024], BF16, tag="at")
                    nc.vector.tensor_scalar_add(st, ps[:, :], qkb[:, f:f + 1])
                    if is_q:
                        nc.sync.dma_start(out=qTd[f], in_=st[:, :])
                    else:
                        nc.sync.dma_start(out=kTd[f - 6, :, tp * 1024:(tp + 1) * 1024],
                                          in_=st[:, :])

            # v (token-major, with ones column at 65-stride for softmax sums)
            vt = {}
            for ts_ in range(NKT):
                ps = psc.tile([P, 1024], F32, tag="sc")
                lt = ts_ // 4
                sub = ts_ % 4
                for co in range(CO):
                    lhs = h1[(co, lt)][:, sub * P:(sub + 1) * P]
                    nc.tensor.matmul(ps[:, 0:512], lhs, wv_sb[:, co, 0:512],
                                     start=(co == 0), stop=(co == CO - 1))
                    nc.tensor.matmul(ps[:, 512:768], lhs, wv_sb[:, co, 512:768],
                                     start=(co == 0), stop=(co == CO - 1))
                va = pvt.tile([P, 12 * 65], BF16, tag="vt")
                va3 = va.rearrange("p (h d) -> p h d", d=65)
                nc.vector.memset(va3[:, :, D:65], 1.0)
                nc.vector.tensor_copy(
                    va3[:, :, 0:D],
                    ps[:, 0:768].rearrange("p (h d) -> p h d", d=D))
                if flags["vbias"]:
                    nc.vector.tensor_tensor(va[:, :], va[:, :], vbb[:, :], OP.add)
                vt[ts_] = va

            # ---------------- attention ------------------------------------
            vals = {}
            for co in range(CO):
                vals[co] = pvl.tile([P, TQ], BF16, tag="vl", name=f"vals{co}")
            for p in range(NPAIR):
                kT = pkq.tile([P, TK], BF16, tag="kq")
                nc.sync.dma_start(out=kT, in_=kTd[p])
                qT = pqt.tile([P, TQ], BF16, tag="qt")
                nc.sync.dma_start(out=qT, in_=qTd[p])
                for qc in range(NQC):
                    pv_ps = [ppv.tile([65, 512], F32, tag="pv", name=f"pv{s_}") for s_ in range(2)]
                    for g in range(8):
                        # issue the s=0 (PE rows 0-63) and s=1 (rows 64-127)
                        # score matmuls back-to-back: the row-disjoint tiles
                        # execute concurrently in the PE array.
                        sc_ps = [psc.tile([P, 1024], F32, tag="sc",
                                          name=f"sc{s_}") for s_ in range(2)]
                        for s in range(2):
                            for ktl in range(2):
                                kt = g * 2 + ktl
                                nc.tensor.matmul(
                                    sc_ps[s][:, ktl * 512:(ktl + 1) * 512],
                                    kT[s * D:(s + 1) * D, kt * P:(kt + 1) * P],
                                    qT[s * D:(s + 1) * D, qc * 512:(qc + 1) * 512],
                                    start=True, stop=True)
                        ats = []
                        for s in range(2):
                            if flags["mask"]:
                                for ktl in range(2):
                                    nc.vector.tensor_tensor(
                                        sc_ps[s][:, ktl * 512:(ktl + 1) * 512],
                                        sc_ps[s][:, ktl * 512:(ktl + 1) * 512],
                                        mqb[:, qc * 512:(qc + 1) * 512], OP.mult)
                            at = pat.tile([P, 1024], BF16, tag="at")
                            nc.scalar.activation(out=at, in_=sc_ps[s][:, :], func=AF.Exp)
                            ats.append(at)
                        for s in range(2):
                            for ktl in range(2):
                                kt = g * 2 + ktl
                                hh = 2 * p + s
                                nc.tensor.matmul(
                                    pv_ps[s][:, :],
                                    vt[kt][:, hh * 65:(hh + 1) * 65],
                                    ats[s][:, ktl * 512:(ktl + 1) * 512],
                                    start=(g == 0 and ktl == 0),
                                    stop=(g == 7 and ktl == 1))
                    for s in range(2):
                        rrow = prow.tile([1, 512], F32, tag="row")
                        nc.vector.reciprocal(out=rrow, in_=pv_ps[s][64:65, :])
                        rb = prb.tile([D, 512], F32, tag="rb")
                        bcast(rb, rrow, D, 512)
                        nc.vector.tensor_tensor(
                            vals[p][s * D:(s + 1) * D, qc * 512:(qc + 1) * 512],
                            pv_ps[s][0:D, :], rb, OP.mult)

            # ---------------- output projection + residual ------------------
            x2 = {}
            for of in range(CO):
                wt = pw.tile([P, CO, P], BF16, tag="w")
                nc.sync.dma_start(out=wt, in_=pjw_d[of])
                ps = psc.tile([P, 1024], F32, tag="sc")
                for co in range(CO):
                    for th in range(2):
                        nc.tensor.matmul(
                            ps[:, th * 512:(th + 1) * 512], wt[:, co, :],
                            vals[co][:, th * 512:(th + 1) * 512],
                            start=(co == 0), stop=(co == CO - 1))
                x2t = px2.tile([P, TQ], BF16, tag="x2")
                for th in range(2):
                    xm = pxy.tile([P, 512], F32, tag="xmy")
                    nc.sync.dma_start(
                        out=xm, in_=xT3[:, of, th * 512:(th + 1) * 512])
                    nc.vector.scalar_tensor_tensor(
                        out=x2t[:, th * 512:(th + 1) * 512],
                        in0=ps[:, th * 512:(th + 1) * 512],
                        scalar=pjb[:, of:of + 1], in1=xm,
                        op0=OP.add, op1=OP.add)
                x2[of] = x2t

            # ---------------- LN2 ------------------------------------------
            h2 = {}
            for t in range(NQC):
                mu_ps = ppv.tile([1, 512], F32, tag="pv")
                m2_ps = ppv.tile([1, 512], F32, tag="pv")
                for co in range(CO):
                    sq = psq.tile([P, 512], BF16, tag="sq")
                    x2c = x2[co][:, t * 512:(t + 1) * 512]
                    nc.vector.tensor_tensor(sq, x2c, x2c, OP.mult)
                    nc.tensor.matmul(mu_ps, ones_b[:, :], x2c,
                                     start=(co == 0), stop=(co == CO - 1))
                    nc.tensor.matmul(m2_ps, ones_b[:, :], sq[:, :],
                                     start=(co == 0), stop=(co == CO - 1))
                mu_row = prow.tile([1, 512], BF16, tag="row")
                nc.vector.tensor_scalar_mul(mu_row, mu_ps, 1.0 / C)
                musq = prow.tile([1, 512], F32, tag="row")
                nc.vector.tensor_tensor(musq, mu_row, mu_row, OP.mult)
                var_row = prow.tile([1, 512], F32, tag="row")
                nc.vector.scalar_tensor_tensor(
                    out=var_row, in0=m2_ps, scalar=1.0 / C, in1=musq,
                    op0=OP.mult, op1=OP.subtract)
                sd_row = prow.tile([1, 512], F32, tag="row")
                nc.scalar.activation(out=sd_row, in_=var_row, func=AF.Sqrt,
                                     bias=eps_sb[0:1, :])
                rs_row = prow.tile([1, 512], F32, tag="row")
                nc.vector.reciprocal(out=rs_row, in_=sd_row)
                mu_b = pst.tile([P, 512], BF16, tag="st")
                bcast(mu_b, mu_row, P, 512)
                rs_b = pst.tile([P, 512], F32, tag="st")
                bcast(rs_b, rs_row, P, 512)
                for co in range(CO):
                    u = pu.tile([P, 512], F32, tag="u")
                    nc.vector.tensor_tensor(u, x2[co][:, t * 512:(t + 1) * 512],
                                            mu_b, OP.subtract)
                    ht = ph2.tile([P, 512], BF16, tag="h2")
                    if flags["ln2_aff"]:
                        nc.vector.tensor_tensor(u, u, rs_b, OP.mult)
                        nc.vector.tensor_scalar(
                            out=ht, in0=u, scalar1=l2g[:, co:co + 1],
                            scalar2=l2b[:, co:co + 1], op0=OP.mult, op1=OP.add)
                    else:
                        nc.vector.tensor_tensor(ht, u, rs_b, OP.mult)
                    h2[(co, t)] = ht

            # ---------------- MLP (token-half split to bound hid SBUF) -----
            for th in range(2):
                hid = {}
                for hf in range(HF):
                    wt = pw.tile([P, CO, P], BF16, tag="w", name=f"w1_{th}_{hf}")
                    nc.sync.dma_start(out=wt, in_=f1w_d[hf])
                    ps = ppv.tile([P, 512], F32, tag="pv", name=f"f1p{th}_{hf}")
                    for co in range(CO):
                        nc.tensor.matmul(ps[:, :], wt[:, co, :],
                                         h2[(co, th)][:, :],
                                         start=(co == 0), stop=(co == CO - 1))
                    g = ph1.tile([P, 512], BF16, tag="h1", name=f"g{th}_{hf}")
                    nc.scalar.activation(out=g, in_=ps[:, :],
                                         func=AF.Gelu, bias=f1b[:, hf:hf + 1])
                    hid[hf] = g

                for of in range(CO):
                    wt2 = pwb.tile([P, HF, P], BF16, tag="wbig",
                                   name=f"w2_{th}_{of}")
                    nc.sync.dma_start(out=wt2, in_=f2w_d[of])
                    ps = psc.tile([P, 1024], F32, tag="sc", name=f"f2p{th}_{of}")
                    for hc in range(HF):
                        nc.tensor.matmul(ps[:, 0:512], wt2[:, hc, :],
                                         hid[hc][:, :],
                                         start=(hc == 0), stop=(hc == HF - 1))
                    ot = pxy.tile([P, 512], F32, tag="xmy", name=f"ot{th}_{of}")
                    nc.vector.scalar_tensor_tensor(
                        out=ot[:, :], in0=ps[:, 0:512],
                        scalar=f2b[:, of:of + 1],
                        in1=x2[of][:, th * 512:(th + 1) * 512],
                        op0=OP.add, op1=OP.add)
                    nc.sync.dma_start(
                        out=out_d[of * P:(of + 1) * P, th * 512:(th + 1) * 512],
                        in_=ot[:, :])

    nc.compile()
    return nc


_CACHE = {}
RUN_KWARGS = {}     # test harness can set {"trace": True}
LAST_RESULT = None  # BassKernelResults of the last kernel() call


def _bf(a):
    return np.ascontiguousarray(a.astype(ml_dtypes.bfloat16))


def _f32(a):
    return np.ascontiguousarray(np.asarray(a, dtype=np.float32))


def kernel(x, mask, ln1_g, ln1_b, qkv_w, qkv_b, proj_w, proj_b,
           ln2_g, ln2_b, fc1_w, fc1_b, fc2_w, fc2_b):
    x = _f32(x); mask = np.asarray(mask)
    ln1_g = _f32(ln1_g); ln1_b = _f32(ln1_b)
    qkv_w = _f32(qkv_w); qkv_b = _f32(qkv_b)
    proj_w = _f32(proj_w); proj_b = _f32(proj_b)
    ln2_g = _f32(ln2_g); ln2_b = _f32(ln2_b)
    fc1_w = _f32(fc1_w); fc1_b = _f32(fc1_b)
    fc2_w = _f32(fc2_w); fc2_b = _f32(fc2_b)
    B, N, Cx = x.shape
    assert (B, N, Cx) == (4, 2048, 768)

    scale = D ** -0.5
    qkv_ws = qkv_w.copy()
    qkv_ws[:, :C] *= scale
    qkv_bs = qkv_b.copy()
    qkv_bs[:C] *= scale

    flags = {
        "ln1_aff": not (np.all(ln1_g == 1) and np.all(ln1_b == 0)),
        "ln2_aff": not (np.all(ln2_g == 1) and np.all(ln2_b == 0)),
        "vbias": not np.all(qkv_bs[2 * C:] == 0),
        "mask": not np.all(mask == 1),
    }

    key = tuple(sorted(flags.items()))
    if key not in _CACHE:
        _CACHE[key] = _build_nc(flags)
    nc = _CACHE[key]

    def tile_lhs(w, nf):
        # w [K, nf*128] -> [nf, 128(ci), K//128(co), 128] contiguous
        K = w.shape[0]
        co = K // P
        r = w.reshape(co, P, nf, P)            # [co, ci, f, j]
        return np.ascontiguousarray(r.transpose(2, 1, 0, 3))  # [f, ci, co, j]

    wqk = _bf(tile_lhs(qkv_ws[:, :2 * C], 12))
    wv = _bf(qkv_ws[:, 2 * C:].reshape(CO, P, C).transpose(1, 0, 2))
    pjw = _bf(tile_lhs(proj_w, CO))
    f1w = _bf(tile_lhs(fc1_w, HF))
    f2w = _bf(tile_lhs(fc2_w, CO))
    qkb = np.ascontiguousarray(qkv_bs[:2 * C].reshape(12, P).T)
    pjb = np.ascontiguousarray(proj_b.reshape(CO, P).T)
    f1b = np.ascontiguousarray(fc1_b.reshape(HF, P).T)
    f2b = np.ascontiguousarray(fc2_b.reshape(CO, P).T)

    shared = {
        "wqk": wqk, "wv": wv, "pjw": pjw, "f1w": f1w, "f2w": f2w,
        "qkb": qkb, "pjb": pjb, "f1b": f1b, "f2b": f2b,
    }
    if flags["ln1_aff"]:
        shared["l1g"] = np.ascontiguousarray(ln1_g.reshape(CO, P).T)
        shared["l1b"] = np.ascontiguousarray(ln1_b.reshape(CO, P).T)
    if flags["ln2_aff"]:
        shared["l2g"] = np.ascontiguousarray(ln2_g.reshape(CO, P).T)
        shared["l2b"] = np.ascontiguousarray(ln2_b.reshape(CO, P).T)
    if flags["vbias"]:
        vb = np.zeros((1, 12 * 65), np.float32)
        vb[0, :].reshape(12, 65)[:, :D] = qkv_bs[2 * C:].reshape(12, D)
        shared["vbb"] = vb

    in_maps = []
    for c in range(8):
        b, half = c // 2, c % 2
        xb = x[b]
        xr = np.concatenate([xb[half * TQ:(half + 1) * TQ],
                             xb[(1 - half) * TQ:(2 - half) * TQ]], axis=0)
        m = dict(shared)
        m["xT"] = np.ascontiguousarray(xr.T)
        if flags["mask"]:
            mr = mask[b].astype(np.float32)[half * TQ:(half + 1) * TQ]
            m["mq"] = np.ascontiguousarray(mr.reshape(1, TQ))
        in_maps.append(m)

    res = run_bass_kernel_spmd(nc, in_maps, core_ids=list(range(8)), **RUN_KWARGS)
    global LAST_RESULT
    LAST_RESULT = res
    out = np.empty((B, N, C), np.float32)
    for c in range(8):
        b, half = c // 2, c % 2
        out[b, half * TQ:(half + 1) * TQ, :] = res.results[c]["outT"].T
    return out



# revision 11
# speedup vs baseline: 1.0403x; 1.0403x over previous
"""Trainium2 Bass kernel for a dense transformer block (LN-attn-LN-MLP).

Sharding: 8 cores = (4 batches) x (2 query-halves). Each core computes k/v for
its batch's full 2048 tokens (duplicated within the pair; avoids collectives)
and queries/MLP for its own 1024 tokens. All activations are kept
feature-major ([C, T]) end-to-end so no on-chip transposes are needed; the
host passes x pre-transposed and un-transposes the output.

All weight GEMMs (qkv, v, proj, fc1, fc2) and the attn*V GEMM run in fp8e4m3
with MatmulPerfMode.DoubleRow (two 128-deep k-tiles contracted per pass).
Weights are pre-scaled by a power of two on the host; the inverse scale is
folded into each PSUM-drain op. Scores stay bf16.

The attention loop runs query-chunk-outer so the full proj/LN2/MLP chain for
query chunk qc overlaps the exp-bound attention of chunk qc+1 on the PE.
"""
import sys
sys.path.insert(0, "/opt/trn_rl_repo")

import math

import numpy as np
import ml_dtypes

import concourse.bass as bass
import concourse.tile as tile
from concourse import bacc
from concourse import mybir
from concourse.bass_utils import run_bass_kernel_spmd

F32 = mybir.dt.float32
BF16 = mybir.dt.bfloat16
F8 = mybir.dt.float8e4
AF = mybir.ActivationFunctionType
OP = mybir.AluOpType
DR = mybir.MatmulPerfMode.DoubleRow

P = 128
C = 768            # embed dim
CO = 6             # C / 128 chunks
H = 12             # heads
D = 64             # head dim
HID = 3072
HF = 24            # HID / 128 chunks
TK = 2048          # tokens per batch (keys/values)
TQ = 1024          # query tokens per core
NKT = TK // P      # 16 key tiles
NTC = TK // 512    # 4 token chunks (LN1)
NQC = TQ // 512    # 2 query chunks
LN_EPS = 1e-6
NPAIR = 6          # head pairs
VROW = 12 * 65     # v_aug row: 12 heads x (64 + ones col)
VPAD = 784         # v_aug pair-dim stride; must be a multiple of 16 for DR


def _build_nc(flags, inv):
    """Build the single-core Bass program (identical for all 8 cores).

    inv: inverse fp8 weight scales {'qk': [12 floats], 'v','pj','f1','f2'}.
    """
    nc = bacc.Bacc()

    xT_d = nc.declare_dram_parameter("xT", [C, TK], F32, isOutput=False)
    wqk_d = nc.declare_dram_parameter("wqk", [12, P, CO, P], F8, isOutput=False)
    wv_d = nc.declare_dram_parameter("wv", [P, CO, C], F8, isOutput=False)
    pjw_d = nc.declare_dram_parameter("pjw", [CO, P, CO, P], BF16, isOutput=False)
    f1w_d = nc.declare_dram_parameter("f1w", [HF, P, CO, P], F8, isOutput=False)
    f2w_d = nc.declare_dram_parameter("f2w", [CO, P, HF, P], BF16, isOutput=False)
    if flags["qkb"]:
        qkb_d = nc.declare_dram_parameter("qkb", [P, 12], F32, isOutput=False)
    if flags["pjb"]:
        pjb_d = nc.declare_dram_parameter("pjb", [P, CO], F32, isOutput=False)
    if flags["f1b"]:
        f1b_d = nc.declare_dram_parameter("f1b", [P, HF], F32, isOutput=False)
    if flags["f2b"]:
        f2b_d = nc.declare_dram_parameter("f2b", [P, CO], F32, isOutput=False)
    if flags["ln1_aff"]:
        l1g_d = nc.declare_dram_parameter("l1g", [P, CO], F32, isOutput=False)
        l1b_d = nc.declare_dram_parameter("l1b", [P, CO], F32, isOutput=False)
    if flags["ln2_aff"]:
        l2g_d = nc.declare_dram_parameter("l2g", [P, CO], F32, isOutput=False)
        l2b_d = nc.declare_dram_parameter("l2b", [P, CO], F32, isOutput=False)
    if flags["vbias"]:
        vbb_d = nc.declare_dram_parameter("vbb", [1, VROW], F32, isOutput=False)
    if flags["mask"]:
        mq_d = nc.declare_dram_parameter("mq", [1, TQ], F32, isOutput=False)
    out_d = nc.declare_dram_parameter("outT", [C, TQ], F32, isOutput=True)

    # internal DRAM spill for q^T / k^T (read back per head-pair)
    kTd = nc.dram_tensor("kT_spill", [NPAIR, P, TK], BF16)
    qTd = nc.dram_tensor("qT_spill", [NPAIR, P, TQ], BF16)

    xT3 = xT_d.rearrange("(co ci) t -> ci co t", ci=P)

    from contextlib import ExitStack
    with tile.TileContext(nc) as tc, ExitStack() as ctx:
        pool = lambda name, bufs, **kw: ctx.enter_context(
            tc.tile_pool(name=name, bufs=bufs, **kw))
        px = pool("px", 2)          # x6 chunks [128,6,512] f32
        psq = pool("psq", 2)        # LN xb/sq [128,512] bf16
        ph1 = pool("ph1", 4)        # h1t [128,6,512] f8 (4 t-chunks resident)
        ph2 = pool("ph2", 2)        # h2t [128,6,512] f8
        phid = pool("phid", 1)      # hid [128,24,512] bf16
        pw = pool("pw", 3)          # [128,6,128] f8 weight tiles
        pwb = pool("pwb", 2)        # wv / fc2 big weight tiles f8
        pqs = pool("pqs", 3)        # qkv drain stage [128,1024] bf16
        pkq = pool("pkq", 2)        # kT per-pair [128,2048] bf16
        pqt = pool("pqt", 2)        # qT per-pair [128,1024] bf16
        pvt = pool("pvt", 8)        # v_aug pairs [128,2,784] f8
        pvl = pool("pvl", 1)        # val6 [128,6,1024] f8
        px2 = pool("px2", 6)        # x2 [128,1024] bf16
        pat = pool("pat", 6)        # attn exp out [128,1024] f8
        pxy = pool("pxy", 3)        # xmy/out [128,512] f32
        pst = pool("pst", 4)        # stat bcast [128,512]
        pu = pool("pu", 2)          # LN centered [128,512] f32
        prow = pool("prow", 4)      # [1,512] rows
        prb = pool("prb", 2)        # recip bcast [64,512] f32
        pone = pool("pone", 1)
        psA = pool("psA", 2, space="PSUM")  # [128,1024] 2-bank slots
        psB = pool("psB", 4, space="PSUM")  # 1-bank slots
        if True:
            ones_b = pone.tile([P, 1], BF16, tag="ones_b")
            nc.vector.memset(ones_b, 1.0)
            eps_sb = pone.tile([P, 1], F32, tag="eps")
            nc.vector.memset(eps_sb, LN_EPS)

            _bcn = [0]

            def bcast(out_ap, row_ap, npart, width):
                """out[0:npart, 0:width] = row broadcast across partitions.
                SBUF APs cannot have partition-step 0, so bounce the row
                through DRAM and broadcast-read it back (two DMAs, no PE)."""
                _bcn[0] += 1
                drow = nc.dram_tensor(f"bcrow{_bcn[0]}", [1, width], row_ap.dtype)
                nc.sync.dma_start(out=drow[:, :], in_=row_ap[0:1, 0:width])
                src = drow[0:1, 0:width]
                bap = bass.AP(tensor=src.tensor, offset=src.offset,
                              ap=[[0, npart]] + list(src.ap[1:]))
                nc.sync.dma_start(out=out_ap[0:npart, 0:width], in_=bap)

            if flags["qkb"]:
                qkb = pone.tile([P, 12], F32, tag="qkb")
                nc.sync.dma_start(out=qkb, in_=qkb_d[:, :])
            if flags["pjb"]:
                pjb = pone.tile([P, CO], F32, tag="pjb")
                nc.sync.dma_start(out=pjb, in_=pjb_d[:, :])
            if flags["f1b"]:
                f1b = pone.tile([P, HF], F32, tag="f1b")
                nc.sync.dma_start(out=f1b, in_=f1b_d[:, :])
            if flags["f2b"]:
                f2b = pone.tile([P, CO], F32, tag="f2b")
                nc.sync.dma_start(out=f2b, in_=f2b_d[:, :])
            if flags["ln1_aff"]:
                l1g = pone.tile([P, CO], F32, tag="l1g")
                nc.sync.dma_start(out=l1g, in_=l1g_d[:, :])
                l1b = pone.tile([P, CO], F32, tag="l1b")
                nc.sync.dma_start(out=l1b, in_=l1b_d[:, :])
            if flags["ln2_aff"]:
                l2g = pone.tile([P, CO], F32, tag="l2g")
                nc.sync.dma_start(out=l2g, in_=l2g_d[:, :])
                l2b = pone.tile([P, CO], F32, tag="l2b")
                nc.sync.dma_start(out=l2b, in_=l2b_d[:, :])
            if flags["vbias"]:
                vbrow = pone.tile([1, VROW], F32, tag="vbrow")
                nc.sync.dma_start(out=vbrow, in_=vbb_d[:, :])
                vbrow_b = pone.tile([1, VROW], BF16, tag="vbrow_b")
                nc.vector.tensor_copy(vbrow_b, vbrow)
                vbb = pone.tile([P, VROW], BF16, tag="vbb")
                bcast(vbb, vbrow_b, P, VROW)
            if flags["mask"]:
                mqrow = pone.tile([1, TQ], F32, tag="mqrow")
                nc.sync.dma_start(out=mqrow, in_=mq_d[:, :])
                mqrow_b = pone.tile([1, TQ], BF16, tag="mqrow_b")
                nc.vector.tensor_copy(mqrow_b, mqrow)
                mqb = pone.tile([P, TQ], BF16, tag="mqb")
                bcast(mqb, mqrow_b, P, TQ)

            wv_sb = pwb.tile([P, CO, C], F8, tag="wbig")
            nc.sync.dma_start(out=wv_sb, in_=wv_d[:, :, :])

            def layer_norm(src_of_co, t, aff, ht):
                """Feature-major LN of one 512-token chunk into ht[:, co, :]."""
                mu_ps = psB.tile([1, 512], F32, tag="pb")
                m2_ps = psB.tile([1, 512], F32, tag="pb")
                srcs = []
                for co in range(CO):
                    s = src_of_co(co)
                    if s.dtype == F32:
                        xb = psq.tile([P, 512], BF16, tag="xb")
                        nc.vector.tensor_copy(xb, s)
                    else:
                        xb = s
                    srcs.append((s, xb))
                    sq = psq.tile([P, 512], BF16, tag="sq")
                    nc.vector.tensor_tensor(sq, xb, xb, OP.mult)
                    nc.tensor.matmul(mu_ps, ones_b[:, :], xb[:, :],
                                     start=(co == 0), stop=(co == CO - 1))
                    nc.tensor.matmul(m2_ps, ones_b[:, :], sq[:, :],
                                     start=(co == 0), stop=(co == CO - 1))
                mu_row = prow.tile([1, 512], BF16, tag="row")
                nc.vector.tensor_scalar_mul(mu_row, mu_ps, 1.0 / C)
                musq = prow.tile([1, 512], F32, tag="row")
                nc.vector.tensor_tensor(musq, mu_row, mu_row, OP.mult)
                var_row = prow.tile([1, 512], F32, tag="row")
                nc.vector.scalar_tensor_tensor(
                    out=var_row, in0=m2_ps, scalar=1.0 / C, in1=musq,
                    op0=OP.mult, op1=OP.subtract)
                sd_row = prow.tile([1, 512], F32, tag="row")
                nc.scalar.activation(out=sd_row, in_=var_row, func=AF.Sqrt,
                                     bias=eps_sb[0:1, :])
                rs_row = prow.tile([1, 512], F32, tag="row")
                nc.vector.reciprocal(out=rs_row, in_=sd_row)
                mu_b = pst.tile([P, 512], BF16, tag="st")
                bcast(mu_b, mu_row, P, 512)
                rs_b = pst.tile([P, 512], F32, tag="st")
                bcast(rs_b, rs_row, P, 512)
                for co in range(CO):
                    s, _ = srcs[co]
                    u = pu.tile([P, 512], F32, tag="u")
                    nc.vector.tensor_tensor(u, s, mu_b, OP.subtract)
                    if aff is not None:
                        g_, b_ = aff
                        nc.vector.tensor_tensor(u, u, rs_b, OP.mult)
                        nc.vector.tensor_scalar(
                            out=ht[:, co, :], in0=u, scalar1=g_[:, co:co + 1],
                            scalar2=b_[:, co:co + 1], op0=OP.mult, op1=OP.add)
                    else:
                        nc.vector.tensor_tensor(ht[:, co, :], u, rs_b, OP.mult)

            # ---------------- LN1 (feature-major, per 512-token chunk) ------
            h1t = {}  # tchunk -> [128, CO, 512] f8
            x6s = {}
            for t in range(NTC):
                x6 = px.tile([P, CO, 512], F32, tag="x6")
                nc.sync.dma_start(out=x6, in_=xT3[:, :, t * 512:(t + 1) * 512])
                x6s[t] = x6
                ht = ph1.tile([P, CO, 512], F8, tag="h1", name=f"h1t{t}")
                aff = (l1g, l1b) if flags["ln1_aff"] else None
                layer_norm(lambda co, x6=x6: x6[:, co, :], t, aff, ht)
                h1t[t] = ht

            # ---------------- QKV projections (fp8 DoubleRow) ---------------
            # q^T / k^T: weight-stationary, feature-major output -> DRAM spill
            for f in range(12):
                is_q = f < 6
                ntp = 1 if is_q else 2   # q: only my 1024 tokens
                wt = pw.tile([P, CO, P], F8, tag="w")
                nc.sync.dma_start(out=wt, in_=wqk_d[f])
                for tp in range(ntp):
                    ps = psA.tile([P, 1024], F32, tag="pa")
                    for jp in range(3):
                        for th in range(2):
                            nc.tensor.matmul(
                                ps[:, th * 512:(th + 1) * 512],
                                wt[:, 2 * jp:2 * jp + 2, :],
                                h1t[tp * 2 + th][:, 2 * jp:2 * jp + 2, :],
                                start=(jp == 0), stop=(jp == 2), perf_mode=DR)
                    st = pqs.tile([P, 1024], BF16, tag="qs")
                    if flags["qkb"]:
                        nc.vector.tensor_scalar(
                            out=st, in0=ps, scalar1=inv["qk"][f],
                            scalar2=qkb[:, f:f + 1], op0=OP.mult, op1=OP.add)
                    else:
                        nc.vector.tensor_scalar_mul(st, ps, inv["qk"][f])
                    if is_q:
                        nc.sync.dma_start(out=qTd[f], in_=st[:, :])
                    else:
                        nc.sync.dma_start(out=kTd[f - 6, :, tp * 1024:(tp + 1) * 1024],
                                          in_=st[:, :])

            # v (token-major, ones col at 65-stride; key-tile PAIRS for DR pv)
            vt = {}
            for g in range(8):
                vp = pvt.tile([P, 2, VPAD], F8, tag="vt", name=f"vp{g}")
                vp3 = vp[:, :, 0:VROW].rearrange("p two (h e) -> p two h e", e=65)
                nc.vector.memset(vp3[:, :, :, D:65], 1.0)
                for ktl in range(2):
                    ts_ = 2 * g + ktl
                    lt = ts_ // 4
                    sub = ts_ % 4
                    ps = psA.tile([P, 1024], F32, tag="pa")
                    for jp in range(3):
                        lhs = h1t[lt][:, 2 * jp:2 * jp + 2, sub * P:(sub + 1) * P]
                        nc.tensor.matmul(ps[:, 0:512], lhs,
                                         wv_sb[:, 2 * jp:2 * jp + 2, 0:512],
                                         start=(jp == 0), stop=(jp == 2),
                                         perf_mode=DR)
                        nc.tensor.matmul(ps[:, 512:768], lhs,
                                         wv_sb[:, 2 * jp:2 * jp + 2, 512:768],
                                         start=(jp == 0), stop=(jp == 2),
                                         perf_mode=DR)
                    nc.vector.tensor_scalar_mul(
                        vp3[:, ktl, :, 0:D],
                        ps[:, 0:768].rearrange("p (h e) -> p h e", e=D),
                        inv["v"])
                    if flags["vbias"]:
                        nc.vector.tensor_tensor(
                            vp[:, ktl, 0:VROW], vp[:, ktl, 0:VROW],
                            vbb[:, :], OP.add)
                vt[g] = vp

            # ------- attention / proj / LN2 / MLP, query-chunk-outer --------
            val6 = pvl.tile([P, CO, TQ], BF16, tag="vl")
            x2 = {of: px2.tile([P, TQ], BF16, tag="x2", name=f"x2_{of}")
                  for of in range(CO)}
            for qc in range(NQC):
                # ---- attention for this query chunk, all head pairs ----
                scol = prow.tile([12, 512], F32, tag="scol", bufs=2)
                for p in range(NPAIR):
                    kT = pkq.tile([P, TK], BF16, tag="kq")
                    nc.sync.dma_start(out=kT, in_=kTd[p])
                    qT = pqt.tile([P, TQ], BF16, tag="qt")
                    nc.sync.dma_start(out=qT, in_=qTd[p])
                    pv_ps = [psB.tile([65, 512], F32, tag="pb", name=f"pv{s_}")
                             for s_ in range(2)]
                    for g in range(8):
                        sc_ps = [psA.tile([P, 1024], F32, tag="pa",
                                          name=f"sc{s_}") for s_ in range(2)]
                        for ktl in range(2):
                            for s in range(2):
                                kt = g * 2 + ktl
                                nc.tensor.matmul(
                                    sc_ps[s][:, ktl * 512:(ktl + 1) * 512],
                                    kT[s * D:(s + 1) * D, kt * P:(kt + 1) * P],
                                    qT[s * D:(s + 1) * D, qc * 512:(qc + 1) * 512],
                                    start=True, stop=True)
                        ats = []
                        for s in range(2):
                            if flags["mask"]:
                                for ktl in range(2):
                                    nc.vector.tensor_tensor(
                                        sc_ps[s][:, ktl * 512:(ktl + 1) * 512],
                                        sc_ps[s][:, ktl * 512:(ktl + 1) * 512],
                                        mqb[:, qc * 512:(qc + 1) * 512], OP.mult)
                            at = pat.tile([P, 1024], F8, tag="at")
                            nc.scalar.activation(out=at, in_=sc_ps[s][:, :],
                                                 func=AF.Exp)
                            ats.append(at)
                        for s in range(2):
                            hh = 2 * p + s
                            at2 = ats[s].rearrange("p (two n) -> p two n", two=2)
                            nc.tensor.matmul(
                                pv_ps[s][:, :],
                                vt[g][:, :, hh * 65:hh * 65 + 65],
                                at2,
                                start=(g == 0), stop=(g == 7), perf_mode=DR)
                    for s in range(2):
                        hh = 2 * p + s
                        # drain unnormalized (scaled) vals; collect sums rows
                        nc.vector.tensor_scalar_mul(
                            val6[s * D:(s + 1) * D, p, qc * 512:(qc + 1) * 512],
                            pv_ps[s][0:D, :], 1.0 / 2048.0)
                        srow = prow.tile([1, 512], F32, tag="row")
                        nc.vector.tensor_scalar_mul(
                            srow, pv_ps[s][64:65, :], 1.0 / 2048.0)
                        nc.sync.dma_start(out=scol[hh:hh + 1, :], in_=srow[:, :])

                rcp = prow.tile([12, 512], F32, tag="rcol", bufs=2)
                nc.vector.reciprocal(out=rcp, in_=scol)
                for p in range(NPAIR):
                    for s in range(2):
                        hh = 2 * p + s
                        rb = prb.tile([P, 512], F32, tag="rb")
                        bcast(rb, rcp[hh:hh + 1, :], P, 512)
                        v6s = val6[s * D:(s + 1) * D, p, qc * 512:(qc + 1) * 512]
                        nc.vector.tensor_tensor(
                            v6s, v6s, rb[s * D:(s + 1) * D, :], OP.mult)

                # ---- output projection + residual for this chunk ----
                for of in range(CO):
                    wt = pw.tile([P, CO, P], BF16, tag="wb")
                    nc.sync.dma_start(out=wt, in_=pjw_d[of])
                    ps = psB.tile([P, 512], F32, tag="pb")
                    for co in range(CO):
                        nc.tensor.matmul(
                            ps[:, :], wt[:, co, :],
                            val6[:, co, qc * 512:(qc + 1) * 512],
                            start=(co == 0), stop=(co == CO - 1))
                    xm = pxy.tile([P, 512], F32, tag="xmy")
                    nc.sync.dma_start(
                        out=xm, in_=xT3[:, of, qc * 512:(qc + 1) * 512])
                    x2s = x2[of][:, qc * 512:(qc + 1) * 512]
                    if flags["pjb"]:
                        nc.vector.scalar_tensor_tensor(
                            out=x2s, in0=ps, scalar=pjb[:, of:of + 1], in1=xm,
                            op0=OP.add, op1=OP.add)
                    else:
                        nc.vector.tensor_tensor(x2s, ps, xm, OP.add)

                # ---- LN2 for this chunk ----
                h2 = ph2.tile([P, CO, 512], F8, tag="h2", name=f"h2t{qc}")
                aff2 = (l2g, l2b) if flags["ln2_aff"] else None
                layer_norm(
                    lambda co: x2[co][:, qc * 512:(qc + 1) * 512], qc, aff2, h2)

                # ---- MLP for this chunk (fp8 DoubleRow) ----
                hid = phid.tile([P, HF, 512], BF16, tag="hid", name=f"hid{qc}")
                for hf in range(HF):
                    wt = pw.tile([P, CO, P], F8, tag="w", name=f"w1_{qc}_{hf}")
                    nc.sync.dma_start(out=wt, in_=f1w_d[hf])
                    ps = psB.tile([P, 512], F32, tag="pb", name=f"f1p{qc}_{hf}")
                    for jp in range(3):
                        nc.tensor.matmul(
                            ps, wt[:, 2 * jp:2 * jp + 2, :],
                            h2[:, 2 * jp:2 * jp + 2, :],
                            start=(jp == 0), stop=(jp == 2), perf_mode=DR)
                    bias = f1b[:, hf:hf + 1] if flags["f1b"] else 0.0
                    nc.scalar.activation(out=hid[:, hf, :], in_=ps,
                                         func=AF.Gelu, bias=bias,
                                         scale=inv["f1"])

                for of in range(CO):
                    wt2 = pwb.tile([P, HF, P], BF16, tag="wbig",
                                   name=f"w2_{qc}_{of}")
                    nc.sync.dma_start(out=wt2, in_=f2w_d[of])
                    ps = psB.tile([P, 512], F32, tag="pb", name=f"f2p{qc}_{of}")
                    for hc in range(HF):
                        nc.tensor.matmul(
                            ps, wt2[:, hc, :], hid[:, hc, :],
                            start=(hc == 0), stop=(hc == HF - 1))
                    ot = pxy.tile([P, 512], F32, tag="xmy", name=f"ot{qc}_{of}")
                    x2s = x2[of][:, qc * 512:(qc + 1) * 512]
                    if flags["f2b"]:
                        nc.vector.scalar_tensor_tensor(
                            out=ot[:, :], in0=ps, scalar=f2b[:, of:of + 1],
                            in1=x2s, op0=OP.add, op1=OP.add)
                    else:
                        nc.vector.tensor_tensor(ot[:, :], ps, x2s, OP.add)
                    nc.sync.dma_start(
                        out=out_d[of * P:(of + 1) * P, qc * 512:(qc + 1) * 512],
                        in_=ot[:, :])

    nc.compile()
    return nc


_CACHE = {}
RUN_KWARGS = {}     # test harness can set {"trace": True}
LAST_RESULT = None  # BassKernelResults of the last kernel() call


def _f32(a):
    return np.ascontiguousarray(np.asarray(a, dtype=np.float32))


def _f8(a):
    return np.ascontiguousarray(
        np.clip(np.asarray(a, np.float32), -448.0, 448.0).astype(
            ml_dtypes.float8_e4m3fn))


def _pow2_scale(absmax):
    """Power-of-two scale putting absmax around 224 (half of e4m3 max)."""
    absmax = float(absmax)
    if absmax <= 0 or not math.isfinite(absmax):
        return 1.0
    return 2.0 ** math.floor(math.log2(224.0 / absmax))


def kernel(x, mask, ln1_g, ln1_b, qkv_w, qkv_b, proj_w, proj_b,
           ln2_g, ln2_b, fc1_w, fc1_b, fc2_w, fc2_b):
    x = _f32(x); mask = np.asarray(mask)
    ln1_g = _f32(ln1_g); ln1_b = _f32(ln1_b)
    qkv_w = _f32(qkv_w); qkv_b = _f32(qkv_b)
    proj_w = _f32(proj_w); proj_b = _f32(proj_b)
    ln2_g = _f32(ln2_g); ln2_b = _f32(ln2_b)
    fc1_w = _f32(fc1_w); fc1_b = _f32(fc1_b)
    fc2_w = _f32(fc2_w); fc2_b = _f32(fc2_b)
    B, N, Cx = x.shape
    assert (B, N, Cx) == (4, 2048, 768)

    scale = D ** -0.5
    qkv_ws = qkv_w.copy()
    qkv_ws[:, :C] *= scale
    qkv_bs = qkv_b.copy()
    qkv_bs[:C] *= scale

    flags = {
        "ln1_aff": not (np.all(ln1_g == 1) and np.all(ln1_b == 0)),
        "ln2_aff": not (np.all(ln2_g == 1) and np.all(ln2_b == 0)),
        "vbias": not np.all(qkv_bs[2 * C:] == 0),
        "mask": not np.all(mask == 1),
        "qkb": not np.all(qkv_bs[:2 * C] == 0),
        "pjb": not np.all(proj_b == 0),
        "f1b": not np.all(fc1_b == 0),
        "f2b": not np.all(fc2_b == 0),
    }

    def tile_lhs(w, nf):
        # w [K, nf*128] -> [nf, 128(ci), K//128(co), 128] contiguous
        K = w.shape[0]
        co = K // P
        r = w.reshape(co, P, nf, P)            # [co, ci, f, j]
        return np.ascontiguousarray(r.transpose(2, 1, 0, 3))  # [f, ci, co, j]

    wqk_t = tile_lhs(qkv_ws[:, :2 * C], 12)
    s_qk = [_pow2_scale(np.max(np.abs(wqk_t[f]))) for f in range(12)]
    wqk = _f8(wqk_t * np.asarray(s_qk, np.float32)[:, None, None, None])
    wv_t = qkv_ws[:, 2 * C:].reshape(CO, P, C).transpose(1, 0, 2)
    s_v = _pow2_scale(np.max(np.abs(wv_t)))
    wv = _f8(wv_t * s_v)
    pjw = np.ascontiguousarray(tile_lhs(proj_w, CO).astype(ml_dtypes.bfloat16))
    f1w_t = tile_lhs(fc1_w, HF)
    s_f1 = _pow2_scale(np.max(np.abs(f1w_t)))
    f1w = _f8(f1w_t * s_f1)
    f2w = np.ascontiguousarray(tile_lhs(fc2_w, CO).astype(ml_dtypes.bfloat16))

    inv = {
        "qk": [1.0 / s for s in s_qk],
        "v": 1.0 / s_v, "f1": 1.0 / s_f1,
    }

    key = (tuple(sorted(flags.items())),
           tuple(s_qk), s_v, s_f1)
    if key not in _CACHE:
        _CACHE[key] = _build_nc(flags, inv)
    nc = _CACHE[key]

    shared = {"wqk": wqk, "wv": wv, "pjw": pjw, "f1w": f1w, "f2w": f2w}
    if flags["qkb"]:
        shared["qkb"] = np.ascontiguousarray(qkv_bs[:2 * C].reshape(12, P).T)
    if flags["pjb"]:
        shared["pjb"] = np.ascontiguousarray(proj_b.reshape(CO, P).T)
    if flags["f1b"]:
        shared["f1b"] = np.ascontiguousarray(fc1_b.reshape(HF, P).T)
    if flags["f2b"]:
        shared["f2b"] = np.ascontiguousarray(fc2_b.reshape(CO, P).T)
    if flags["ln1_aff"]:
        shared["l1g"] = np.ascontiguousarray(ln1_g.reshape(CO, P).T)
        shared["l1b"] = np.ascontiguousarray(ln1_b.reshape(CO, P).T)
    if flags["ln2_aff"]:
        shared["l2g"] = np.ascontiguousarray(ln2_g.reshape(CO, P).T)
        shared["l2b"] = np.ascontiguousarray(ln2_b.reshape(CO, P).T)
    if flags["vbias"]:
        vb = np.zeros((1, VROW), np.float32)
        vb[0, :].reshape(12, 65)[:, :D] = qkv_bs[2 * C:].reshape(12, D)
        shared["vbb"] = vb

    in_maps = []
    for c in range(8):
        b, half = c // 2, c % 2
        xb = x[b]
        xr = np.concatenate([xb[half * TQ:(half + 1) * TQ],
                             xb[(1 - half) * TQ:(2 - half) * TQ]], axis=0)
        m = dict(shared)
        m["xT"] = np.ascontiguousarray(xr.T)
        if flags["mask"]:
            mr = mask[b].astype(np.float32)[half * TQ:(half + 1) * TQ]
            m["mq"] = np.ascontiguousarray(mr.reshape(1, TQ))
        in_maps.append(m)

    res = run_bass_kernel_spmd(nc, in_maps, core_ids=list(range(8)), **RUN_KWARGS)
    global LAST_RESULT
    LAST_RESULT = res
    out = np.empty((B, N, C), np.float32)
    for c in range(8):
        b, half = c // 2, c % 2
        out[b, half * TQ:(half + 1) * TQ, :] = res.results[c]["outT"].T
    return out
